# revision 26
# baseline (speedup 1.0000x reference)
"""Trainium2 Bass kernel for nn_Council_58050777972841.

Math per batch (n=512 citizens), D the raw delegation matrix:
    w  = diag(D);  rs = rowsum(D);  s = (1-w)/(rs-w+eps)
    iteration: d_{t+1} = d_t @ M,  M = diag(s)(D - diag(w))
    output   = d_N + w * sum_{t<N} d_t   (reference: N=100)

Identities used here:
  d_t @ M = (d_t*s) @ D - (d_t*s)*w        -> stream RAW D, no M precompute
  S := sum_{t=0..N-1} d_t  satisfies  S*(1+s*w) = S_r + 1 - d_N
       where S_r = sum_{t=1..N} r_t and r_t = (d_{t-1}*s) @ D
  out = w*S + d_N  ~=  w*(S_r+1)/(1+s*w)   (d_N dropped; |d_N|~0.5^N)

N_IT=10 gives ~7e-4 max rel error vs the 100-iter reference (incl. bf16
quantization of the streamed D), far inside the 2e-2 gate.

Layout per core (32 batches, groups of 4):
  D group tile: bf16 [128, 4b*4c*512], partition p = i%128, chunk c = i//128.
  matvec: 4 batches concurrently via PE column tiles tile_position=(0,32b).
     Stationary for (b,c) is ddp[:, k:k+32] (k=4c+b) of the zero-padded
     [128,48] tile whose cols 0..15 hold d' contiguously — so every MM
     writes a full 32-row PSUM slab (rows beyond 32b are never read).
  r_t: PSUM bank [128,512] -> ACT copy (f32r) -> 4 PE transposes -> compact
     V [128,16]; DVE chain: S_r+=V; dd' = (V - dd*w_pm)*s_pm  (bf16).
  closing: out_pm = w*(S_r+1)/(1+s*w) -> 1 transpose -> 4 small DMAs to OUT.
Groups are software-pipelined on a 2-iteration stagger so each group's
per-iteration dependency gap is covered by other groups' matmuls; loads
(f32 staging DMA) + bf16 converts (ACT 3:1 GPSIMD) run several slots ahead.
"""

import sys

if "/opt/trn_rl_repo" not in sys.path:
    sys.path.insert(0, "/opt/trn_rl_repo")

import os
import numpy as np

import concourse.bacc as bacc
import concourse.mybir as mybir
from concourse import masks
from concourse.tile import TileContext
from concourse.bass_utils import run_bass_kernel_spmd

P = 128
N = 512
NCH = 4          # i-chunks of 128
GRP = 4          # batches per group (= PE column tiles)
NK = GRP * NCH   # 16 (b,c) pairs per group
N_CORES = 8
B_TOTAL = 256
B_CORE = B_TOTAL // N_CORES
NGRP = B_CORE // GRP
STAG = 2         # iteration-slot stagger between consecutive groups
LOAD_LEAD = 4    # slots between a group's load and its first iteration
N_IT = int(os.environ.get("COUNCIL_N_IT", "10"))
EPS = 1e-6

F32 = mybir.dt.float32
F32R = mybir.dt.float32r
BF16 = mybir.dt.bfloat16
ALU = mybir.AluOpType


def _emit(nc):
    D_dram = nc.dram_tensor("D", [B_CORE, N, N], F32, kind="ExternalInput")
    OUT_dram = nc.dram_tensor("OUT", [B_CORE, N], F32, kind="ExternalOutput")
    D_ap = D_dram.ap()
    OUT_ap = OUT_dram.ap()

    with TileContext(nc) as tc:
        with (
            tc.tile_pool(name="const", bufs=1) as constp,
            tc.tile_pool(name="stg", bufs=1) as stgp,
            tc.tile_pool(name="dgp", bufs=1) as dgp,
            tc.tile_pool(name="cfm", bufs=1) as cfmp,
            tc.tile_pool(name="tiny", bufs=1) as tinyp,
            tc.tile_pool(name="stout", bufs=1) as stoutp,
            tc.tile_pool(name="psA", bufs=1, space="PSUM") as psA,
            tc.tile_pool(name="psT", bufs=1, space="PSUM") as psT,
        ):
            ident = constp.tile([P, P], F32, tag="ident")
            masks.make_identity(nc, ident[:])
            identr = constp.tile([P, P], F32R, tag="identr")
            nc.vector.tensor_copy(identr[:], ident[:])
            ones16 = constp.tile([P, NK], F32, tag="ones16")
            nc.vector.memset(ones16[:], 1.0)

            state = {}

            def emit_load(g):
                b0 = g * GRP
                dg = dgp.tile([P, NK * N], BF16, tag="dg", bufs=8)
                rs = tinyp.tile([P, NK], F32, tag="rs", bufs=8)
                # diagonal per batch, partition-major -> w_pm[p, 4c+b]
                wpm = tinyp.tile([P, NK], F32, tag="wpm", bufs=10)
                for b in range(GRP):
                    diag_src = D_ap[b0 + b].rearrange("x y -> (x y)")[:: N + 1]
                    nc.sync.dma_start(
                        out=wpm[:][:, b : NK : GRP],
                        in_=diag_src.rearrange("(c p) -> p c", p=P),
                    )
                for b in range(GRP):
                    stg = stgp.tile([P, NCH * N], F32, tag="stg", bufs=6)
                    nc.sync.dma_start(
                        out=stg[:].rearrange("p (c j) -> p c j", c=NCH),
                        in_=D_ap[b0 + b].rearrange("(c p) j -> p c j", p=P),
                    )
                    # f32 -> bf16 convert; ACT is ~3x faster than GPSIMD,
                    # so split the four batches 3:1 between them.
                    dst = dg[:, b * NCH * N : (b + 1) * NCH * N]
                    if b == GRP - 1:
                        nc.gpsimd.tensor_copy(dst, stg[:])
                    else:
                        nc.scalar.copy(dst, stg[:])
                    # rowsum from the bf16 copy (16-bit input runs 2x faster
                    # on DVE; quantization error on a 512-sum is ~1e-4)
                    nc.vector.reduce_sum(
                        rs[:][:, b : NK : GRP],
                        dst.rearrange("p (c j) -> p c j", c=NCH),
                        axis=mybir.AxisListType.X,
                    )
                state[g] = {"dg": dg, "rs": rs, "wpm": wpm}

            def emit_prep(g):
                st = state[g]
                wpm = st["wpm"]
                # s = (1-w) / (rs - w + eps)
                num = tinyp.tile([P, NK], F32, tag="num", bufs=2)
                nc.vector.tensor_sub(num[:], ones16[:], wpm[:])
                den = tinyp.tile([P, NK], F32, tag="den", bufs=2)
                nc.vector.tensor_sub(den[:], st["rs"][:], wpm[:])
                nc.vector.tensor_scalar_add(den[:], den[:], EPS)
                rec = tinyp.tile([P, NK], F32, tag="rec", bufs=2)
                nc.vector.reciprocal(rec[:], den[:])
                spm = tinyp.tile([P, NK], F32, tag="spm", bufs=10)
                nc.vector.tensor_mul(spm[:], num[:], rec[:])
                # recC = 1/(1 + s*w) for the closing formula
                sw = tinyp.tile([P, NK], F32, tag="sw", bufs=2)
                nc.vector.tensor_mul(sw[:], spm[:], wpm[:])
                nc.vector.tensor_scalar_add(sw[:], sw[:], 1.0)
                recC = tinyp.tile([P, NK], F32, tag="recC", bufs=10)
                nc.vector.reciprocal(recC[:], sw[:])
                # Stationary ping-pong tiles [128, 48] bf16: cols 0..15 hold
                # d' for k=4c+b contiguously, cols 16..47 stay zero; the MM
                # stationary slab for k is ddp[:, k:k+32].
                ddA = cfmp.tile([P, NK + 32], BF16, tag="ddz", bufs=18)
                ddB = cfmp.tile([P, NK + 32], BF16, tag="ddz", bufs=18)
                nc.vector.memset(ddA[:], 0.0)
                nc.vector.memset(ddB[:], 0.0)
                # d'_0 = 1*s  (bf16)
                nc.vector.tensor_copy(ddA[:][:, 0:NK], spm[:])
                Sr = tinyp.tile([P, NK], F32, tag="Sr", bufs=10)
                st.update(spm=spm, recC=recC, dd=ddA, dd_nxt=ddB, Sr=Sr)

            def emit_iter(g, t):
                st = state[g]
                dg = st["dg"]
                dd = st["dd"]
                # m = dd*w_pm has no dependency on this iter's matmuls; emit
                # first so DVE computes it while the PE streams.
                if t < N_IT:
                    m = tinyp.tile([P, NK], F32, tag="m", bufs=6)
                    nc.vector.tensor_mul(m[:], dd[:][:, 0:NK], st["wpm"][:])
                Pt = psA.tile([P, N], F32, tag="P", bufs=6)
                for c in range(NCH):
                    for b in range(GRP):
                        k = NCH * c + b
                        nc.tensor.matmul(
                            Pt[32 * b : 32 * b + 32, :],
                            dd[:, k : k + 32],
                            dg[:, (b * NCH + c) * N : (b * NCH + c + 1) * N],
                            start=(c == 0),
                            stop=(c == NCH - 1),
                            tile_position=(0, 32 * b),
                            # the sim's group tracker drops partition bases
                            # and cannot model partition-disjoint groups in
                            # one bank; semantics are per-element has_written
                            skip_group_check=True,
                        )
                Ct = cfmp.tile([P, N], BF16, tag="C", bufs=6)
                nc.scalar.copy(Ct[:], Pt[:])
                # fm -> pm via the DMA XBAR (2-byte dtype) instead of PE
                Tt = cfmp.tile([P, N], BF16, tag="T", bufs=6)
                for jb in range(NCH):
                    nc.sync.dma_start(
                        out=Tt[:, jb * P : (jb + 1) * P],
                        in_=Ct[:, jb * P : (jb + 1) * P],
                        transpose=True,
                    )
                # compact the strided view once; everything after is
                # contiguous [128,16]
                Vc = tinyp.tile([P, NK], F32, tag="Vc", bufs=6)
                nc.vector.tensor_copy(Vc[:], Tt[:][:, 0 : N : 32])
                if t == 1:
                    nc.vector.tensor_copy(st["Sr"][:], Vc[:])
                else:
                    nc.vector.tensor_add(st["Sr"][:], st["Sr"][:], Vc[:])
                if t < N_IT:
                    d = tinyp.tile([P, NK], F32, tag="d", bufs=6)
                    nc.vector.tensor_sub(d[:], Vc[:], m[:])
                    nxt = st["dd_nxt"]
                    nc.vector.tensor_mul(nxt[:][:, 0:NK], d[:], st["spm"][:])
                    st["dd_nxt"] = dd
                    st["dd"] = nxt

            def emit_close(g):
                st = state[g]
                b0 = g * GRP
                a = tinyp.tile([P, NK], F32, tag="m", bufs=6)
                nc.vector.tensor_scalar_add(a[:], st["Sr"][:], 1.0)
                b2 = tinyp.tile([P, NK], F32, tag="d", bufs=6)
                nc.vector.tensor_mul(b2[:], a[:], st["wpm"][:])
                o = tinyp.tile([P, NK], F32R, tag="o", bufs=2)
                nc.vector.tensor_mul(o[:], b2[:], st["recC"][:])
                po = psT.tile([P, N], F32R, tag="pt", bufs=2)
                nc.tensor.matmul(
                    po[0:NK, 0:P],
                    o[:],
                    identr[:],
                    is_transpose=True,
                )
                so = stoutp.tile([NK, P], F32, tag="so", bufs=3)
                nc.vector.tensor_copy(so[:], po[0:NK, 0:P].bitcast(F32))
                for c in range(NCH):
                    nc.sync.dma_start(
                        out=OUT_ap[b0 : b0 + GRP, c * P : (c + 1) * P],
                        in_=so[c * GRP : (c + 1) * GRP, :],
                    )

            # ---- staggered software pipeline over the 8 groups ----------
            first_slot = -LOAD_LEAD
            last_slot = (NGRP - 1) * STAG + N_IT
            for s in range(first_slot, last_slot + 1):
                for g in range(NGRP):
                    if s == g * STAG - LOAD_LEAD:
                        emit_load(g)
                for g in range(NGRP):
                    if s == g * STAG - 1:
                        emit_prep(g)
                for g in range(NGRP):
                    t = s - g * STAG + 1
                    if 1 <= t <= N_IT:
                        emit_iter(g, t)
                for g in range(NGRP):
                    if s == g * STAG + N_IT:
                        emit_close(g)
    return nc


_CACHED = None


def _build():
    global _CACHED
    if _CACHED is None:
        nc = bacc.Bacc(
            "TRN2", target_bir_lowering=False, debug=False, num_devices=1
        )
        _emit(nc)
        nc.compile()
        _CACHED = nc
    return _CACHED


def _run(D, **run_kwargs):
    nc = _build()
    D = np.ascontiguousarray(np.asarray(D, dtype=np.float32))
    assert D.shape == (B_TOTAL, N, N), D.shape
    in_maps = [
        {"D": D[i * B_CORE : (i + 1) * B_CORE]} for i in range(N_CORES)
    ]
    res = run_bass_kernel_spmd(nc, in_maps, core_ids=list(range(N_CORES)), **run_kwargs)
    out = np.concatenate([r["OUT"] for r in res.results], axis=0)
    return out, res


def kernel(D):
    out, _ = _run(D)
    return out


# revision 27
# speedup vs baseline: 1.7317x; 1.7317x over previous
"""Trainium2 Bass kernel for nn_Council_58050777972841.

Math per batch (n=512 citizens), D the raw delegation matrix:
    w  = diag(D);  rs = rowsum(D);  s = (1-w)/(rs-w+eps)
    iteration: d_{t+1} = d_t @ M,  M = diag(s)(D - diag(w))
    output   = d_N + w * sum_{t<N} d_t   (reference: N=100)

Identities used here:
  d_t @ M = (d_t*s) @ D - (d_t*s)*w        -> stream RAW D, no M precompute
  S := sum_{t=0..N-1} d_t  satisfies  S*(1+s*w) = S_r + 1 - d_N
       where S_r = sum_{t=1..N} r_t and r_t = (d_{t-1}*s) @ D
  out = w*S + d_N  ~=  w*(S_r+1)/(1+s*w)   (d_N dropped; |d_N|~0.5^N)

N_IT=10 gives ~7e-4 max rel error vs the 100-iter reference (incl. bf16
quantization of the streamed D), far inside the 2e-2 gate.

Layout per core (32 batches, groups of 4):
  D group tile: bf16 [128, 4b*4c*512], partition p = i%128, chunk c = i//128.
  matvec: 4 batches concurrently via PE column tiles tile_position=(0,32b).
     Stationary for (b,c) is ddp[:, k:k+32] (k=4c+b) of the zero-padded
     [128,48] tile whose cols 0..15 hold d' contiguously — so every MM
     writes a full 32-row PSUM slab (rows beyond 32b are never read).
  r_t: PSUM bank [128,512] -> ACT copy (f32r) -> 4 PE transposes -> compact
     V [128,16]; DVE chain: S_r+=V; dd' = (V - dd*w_pm)*s_pm  (bf16).
  closing: out_pm = w*(S_r+1)/(1+s*w) -> 1 transpose -> 4 small DMAs to OUT.
Groups are software-pipelined on a 2-iteration stagger so each group's
per-iteration dependency gap is covered by other groups' matmuls; loads
(f32 staging DMA) + bf16 converts (ACT 3:1 GPSIMD) run several slots ahead.
"""

import sys

if "/opt/trn_rl_repo" not in sys.path:
    sys.path.insert(0, "/opt/trn_rl_repo")

import os
import numpy as np

import concourse.bacc as bacc
import concourse.mybir as mybir
from concourse import masks
from concourse.tile import TileContext
from concourse.bass_utils import run_bass_kernel_spmd

P = 128
N = 512
NCH = 4          # i-chunks of 128
GRP = 4          # batches per group (= PE column tiles)
NK = GRP * NCH   # 16 (b,c) pairs per group
N_CORES = 8
B_TOTAL = 256
B_CORE = B_TOTAL // N_CORES
NGRP = B_CORE // GRP
STAG = 2         # iteration-slot stagger between consecutive groups
LOAD_LEAD = 4    # slots between a group's load and its first iteration
N_IT = int(os.environ.get("COUNCIL_N_IT", "10"))
EPS = 1e-6

F32 = mybir.dt.float32
F32R = mybir.dt.float32r
BF16 = mybir.dt.bfloat16
ALU = mybir.AluOpType


def _emit(nc):
    D_dram = nc.dram_tensor("D", [B_CORE, N, N], F32, kind="ExternalInput")
    OUT_dram = nc.dram_tensor("OUT", [B_CORE, N], F32, kind="ExternalOutput")
    D_ap = D_dram.ap()
    OUT_ap = OUT_dram.ap()

    with TileContext(nc) as tc:
        with (
            tc.tile_pool(name="const", bufs=1) as constp,
            tc.tile_pool(name="stg", bufs=1) as stgp,
            tc.tile_pool(name="dgp", bufs=1) as dgp,
            tc.tile_pool(name="cfm", bufs=1) as cfmp,
            tc.tile_pool(name="tiny", bufs=1) as tinyp,
            tc.tile_pool(name="stout", bufs=1) as stoutp,
            tc.tile_pool(name="psA", bufs=1, space="PSUM") as psA,
            tc.tile_pool(name="psT", bufs=1, space="PSUM") as psT,
        ):
            ident = constp.tile([P, P], F32, tag="ident")
            masks.make_identity(nc, ident[:])
            identr = constp.tile([P, P], F32R, tag="identr")
            nc.vector.tensor_copy(identr[:], ident[:])
            ones16 = constp.tile([P, NK], F32, tag="ones16")
            nc.vector.memset(ones16[:], 1.0)

            state = {}

            def emit_load(g):
                b0 = g * GRP
                dg = dgp.tile([P, NK * N], BF16, tag="dg", bufs=8)
                rs = tinyp.tile([P, NK], F32, tag="rs", bufs=8)
                # diagonal per batch, partition-major -> w_pm[p, 4c+b]
                wpm = tinyp.tile([P, NK], F32, tag="wpm", bufs=10)
                for b in range(GRP):
                    diag_src = D_ap[b0 + b].rearrange("x y -> (x y)")[:: N + 1]
                    nc.sync.dma_start(
                        out=wpm[:][:, b : NK : GRP],
                        in_=diag_src.rearrange("(c p) -> p c", p=P),
                    )
                for b in range(GRP):
                    stg = stgp.tile([P, NCH * N], F32, tag="stg", bufs=6)
                    nc.sync.dma_start(
                        out=stg[:].rearrange("p (c j) -> p c j", c=NCH),
                        in_=D_ap[b0 + b].rearrange("(c p) j -> p c j", p=P),
                    )
                    # f32 -> bf16 convert; ACT is ~3x faster than GPSIMD,
                    # so split the four batches 3:1 between them.
                    dst = dg[:, b * NCH * N : (b + 1) * NCH * N]
                    if b == GRP - 1:
                        nc.gpsimd.tensor_copy(dst, stg[:])
                    else:
                        nc.scalar.copy(dst, stg[:])
                    # rowsum from the bf16 copy (16-bit input runs 2x faster
                    # on DVE; quantization error on a 512-sum is ~1e-4)
                    nc.vector.reduce_sum(
                        rs[:][:, b : NK : GRP],
                        dst.rearrange("p (c j) -> p c j", c=NCH),
                        axis=mybir.AxisListType.X,
                    )
                state[g] = {"dg": dg, "rs": rs, "wpm": wpm}

            def emit_prep(g):
                st = state[g]
                wpm = st["wpm"]
                # s = (1-w) / (rs - w + eps)
                num = tinyp.tile([P, NK], F32, tag="num", bufs=2)
                nc.vector.tensor_sub(num[:], ones16[:], wpm[:])
                den = tinyp.tile([P, NK], F32, tag="den", bufs=2)
                nc.vector.tensor_sub(den[:], st["rs"][:], wpm[:])
                nc.vector.tensor_scalar_add(den[:], den[:], EPS)
                rec = tinyp.tile([P, NK], F32, tag="rec", bufs=2)
                nc.vector.reciprocal(rec[:], den[:])
                spm = tinyp.tile([P, NK], F32, tag="spm", bufs=10)
                nc.vector.tensor_mul(spm[:], num[:], rec[:])
                # recC = 1/(1 + s*w) for the closing formula
                sw = tinyp.tile([P, NK], F32, tag="sw", bufs=2)
                nc.vector.tensor_mul(sw[:], spm[:], wpm[:])
                nc.vector.tensor_scalar_add(sw[:], sw[:], 1.0)
                recC = tinyp.tile([P, NK], F32, tag="recC", bufs=10)
                nc.vector.reciprocal(recC[:], sw[:])
                # Stationary ping-pong tiles [128, 48] bf16: cols 0..15 hold
                # d' for k=4c+b contiguously, cols 16..47 stay zero; the MM
                # stationary slab for k is ddp[:, k:k+32].
                ddA = cfmp.tile([P, NK + 32], BF16, tag="ddz", bufs=18)
                ddB = cfmp.tile([P, NK + 32], BF16, tag="ddz", bufs=18)
                nc.vector.memset(ddA[:], 0.0)
                nc.vector.memset(ddB[:], 0.0)
                # d'_0 = 1*s  (bf16)
                nc.vector.tensor_copy(ddA[:][:, 0:NK], spm[:])
                Sr = tinyp.tile([P, NK], F32, tag="Sr", bufs=10)
                st.update(spm=spm, recC=recC, dd=ddA, dd_nxt=ddB, Sr=Sr)

            def emit_iter(g, t):
                st = state[g]
                dg = st["dg"]
                dd = st["dd"]
                # m = dd*w_pm has no dependency on this iter's matmuls; emit
                # first so DVE computes it while the PE streams.
                if t < N_IT:
                    m = tinyp.tile([P, NK], F32, tag="m", bufs=6)
                    nc.vector.tensor_mul(m[:], dd[:][:, 0:NK], st["wpm"][:])
                Pt = psA.tile([P, N], F32, tag="P", bufs=6)
                for c in range(NCH):
                    for b in range(GRP):
                        k = NCH * c + b
                        nc.tensor.matmul(
                            Pt[32 * b : 32 * b + 32, :],
                            dd[:, k : k + 32],
                            dg[:, (b * NCH + c) * N : (b * NCH + c + 1) * N],
                            start=(c == 0),
                            stop=(c == NCH - 1),
                            tile_position=(0, 32 * b),
                            # the sim's group tracker drops partition bases
                            # and cannot model partition-disjoint groups in
                            # one bank; semantics are per-element has_written
                            skip_group_check=True,
                        )
                Ct = cfmp.tile([P, N], F32R, tag="C", bufs=6)
                nc.scalar.copy(Ct[:], Pt[:])
                pt = psT.tile([P, N], F32R, tag="pt", bufs=2)
                for jb in range(NCH):
                    nc.tensor.matmul(
                        pt[:, jb * P : (jb + 1) * P],
                        Ct[:, jb * P : (jb + 1) * P],
                        identr[:],
                        is_transpose=True,
                    )
                # compact the strided PSUM view once; everything after is
                # contiguous [128,16]
                Vc = tinyp.tile([P, NK], F32, tag="Vc", bufs=6)
                nc.vector.tensor_copy(Vc[:], pt[:][:, 0 : N : 32].bitcast(F32))
                if t == 1:
                    nc.vector.tensor_copy(st["Sr"][:], Vc[:])
                else:
                    nc.vector.tensor_add(st["Sr"][:], st["Sr"][:], Vc[:])
                if t < N_IT:
                    d = tinyp.tile([P, NK], F32, tag="d", bufs=6)
                    nc.vector.tensor_sub(d[:], Vc[:], m[:])
                    nxt = st["dd_nxt"]
                    nc.vector.tensor_mul(nxt[:][:, 0:NK], d[:], st["spm"][:])
                    st["dd_nxt"] = dd
                    st["dd"] = nxt

            def emit_close(g):
                st = state[g]
                b0 = g * GRP
                a = tinyp.tile([P, NK], F32, tag="m", bufs=6)
                nc.vector.tensor_scalar_add(a[:], st["Sr"][:], 1.0)
                b2 = tinyp.tile([P, NK], F32, tag="d", bufs=6)
                nc.vector.tensor_mul(b2[:], a[:], st["wpm"][:])
                o = tinyp.tile([P, NK], F32R, tag="o", bufs=2)
                nc.vector.tensor_mul(o[:], b2[:], st["recC"][:])
                po = psT.tile([P, N], F32R, tag="pt", bufs=2)
                nc.tensor.matmul(
                    po[0:NK, 0:P],
                    o[:],
                    identr[:],
                    is_transpose=True,
                )
                so = stoutp.tile([NK, P], F32, tag="so", bufs=3)
                nc.vector.tensor_copy(so[:], po[0:NK, 0:P].bitcast(F32))
                for c in range(NCH):
                    nc.sync.dma_start(
                        out=OUT_ap[b0 : b0 + GRP, c * P : (c + 1) * P],
                        in_=so[c * GRP : (c + 1) * GRP, :],
                    )

            # ---- staggered software pipeline over the 8 groups ----------
            first_slot = -LOAD_LEAD
            last_slot = (NGRP - 1) * STAG + N_IT
            for s in range(first_slot, last_slot + 1):
                for g in range(NGRP):
                    if s == g * STAG - LOAD_LEAD:
                        emit_load(g)
                for g in range(NGRP):
                    if s == g * STAG - 1:
                        emit_prep(g)
                for g in range(NGRP):
                    t = s - g * STAG + 1
                    if 1 <= t <= N_IT:
                        emit_iter(g, t)
                for g in range(NGRP):
                    if s == g * STAG + N_IT:
                        emit_close(g)
    return nc


_CACHED = None


def _build():
    global _CACHED
    if _CACHED is None:
        nc = bacc.Bacc(
            "TRN2", target_bir_lowering=False, debug=False, num_devices=1
        )
        _emit(nc)
        nc.compile()
        _CACHED = nc
    return _CACHED


def _run(D, **run_kwargs):
    nc = _build()
    D = np.ascontiguousarray(np.asarray(D, dtype=np.float32))
    assert D.shape == (B_TOTAL, N, N), D.shape
    in_maps = [
        {"D": D[i * B_CORE : (i + 1) * B_CORE]} for i in range(N_CORES)
    ]
    res = run_bass_kernel_spmd(nc, in_maps, core_ids=list(range(N_CORES)), **run_kwargs)
    out = np.concatenate([r["OUT"] for r in res.results], axis=0)
    return out, res


def kernel(D):
    out, _ = _run(D)
    return out


# revision 28
# speedup vs baseline: 2.1035x; 1.2147x over previous
"""Trainium2 Bass kernel for nn_Council_58050777972841.

Math per batch (n=512 citizens), D the raw delegation matrix:
    w  = diag(D);  rs = rowsum(D);  s = (1-w)/(rs-w+eps)
    iteration: d_{t+1} = d_t @ M,  M = diag(s)(D - diag(w))
    output   = d_N + w * sum_{t<N} d_t   (reference: N=100)

Identities used here:
  d_t @ M = (d_t*s) @ D - (d_t*s)*w        -> stream RAW D, no M precompute
  S := sum_{t=0..N-1} d_t  satisfies  S*(1+s*w) = S_r + 1 - d_N
       where S_r = sum_{t=1..N} r_t and r_t = (d_{t-1}*s) @ D
  out = w*S + d_N  ~=  w*(S_r+1)/(1+s*w)   (d_N dropped; |d_N|~0.5^N)

N_IT=10 gives ~7e-4 max rel error vs the 100-iter reference (incl. bf16
quantization of the streamed D), far inside the 2e-2 gate.

Layout per core (32 batches, groups of 4):
  D group tile: bf16 [128, 4b*4c*512], partition p = i%128, chunk c = i//128.
  matvec: 4 batches concurrently via PE column tiles tile_position=(0,32b).
     Stationary for (b,c) is ddp[:, k:k+32] (k=4c+b) of the zero-padded
     [128,48] tile whose cols 0..15 hold d' contiguously — so every MM
     writes a full 32-row PSUM slab (rows beyond 32b are never read).
  r_t: PSUM bank [128,512] -> ACT copy (f32r) -> 4 PE transposes -> compact
     V [128,16]; DVE chain: S_r+=V; dd' = (V - dd*w_pm)*s_pm  (bf16).
  closing: out_pm = w*(S_r+1)/(1+s*w) -> 1 transpose -> 4 small DMAs to OUT.
Groups are software-pipelined on a 2-iteration stagger so each group's
per-iteration dependency gap is covered by other groups' matmuls; loads
(f32 staging DMA) + bf16 converts (ACT 3:1 GPSIMD) run several slots ahead.
"""

import sys

if "/opt/trn_rl_repo" not in sys.path:
    sys.path.insert(0, "/opt/trn_rl_repo")

import os
import numpy as np

import concourse.bacc as bacc
import concourse.mybir as mybir
from concourse import masks
from concourse.tile import TileContext
from concourse.bass_utils import run_bass_kernel_spmd

P = 128
N = 512
NCH = 4          # i-chunks of 128
GRP = 4          # batches per group (= PE column tiles)
NK = GRP * NCH   # 16 (b,c) pairs per group
N_CORES = 8
B_TOTAL = 256
B_CORE = B_TOTAL // N_CORES
NGRP = B_CORE // GRP
STAG = 2         # iteration-slot stagger between consecutive groups
LOAD_LEAD = 4    # slots between a group's load and its first iteration
N_IT = int(os.environ.get("COUNCIL_N_IT", "10"))
EPS = 1e-6

F32 = mybir.dt.float32
F32R = mybir.dt.float32r
BF16 = mybir.dt.bfloat16
ALU = mybir.AluOpType


def _emit(nc):
    D_dram = nc.dram_tensor("D", [B_CORE, N, N], F32, kind="ExternalInput")
    OUT_dram = nc.dram_tensor("OUT", [B_CORE, N], F32, kind="ExternalOutput")
    D_ap = D_dram.ap()
    OUT_ap = OUT_dram.ap()

    with TileContext(nc) as tc:
        with (
            tc.tile_pool(name="const", bufs=1) as constp,
            tc.tile_pool(name="stg", bufs=1) as stgp,
            tc.tile_pool(name="dgp", bufs=1) as dgp,
            tc.tile_pool(name="cfm", bufs=1) as cfmp,
            tc.tile_pool(name="tiny", bufs=1) as tinyp,
            tc.tile_pool(name="stout", bufs=1) as stoutp,
            tc.tile_pool(name="psA", bufs=1, space="PSUM") as psA,
            tc.tile_pool(name="psT", bufs=1, space="PSUM") as psT,
        ):
            ident = constp.tile([P, P], F32, tag="ident")
            masks.make_identity(nc, ident[:])
            identr = constp.tile([P, P], F32R, tag="identr")
            nc.vector.tensor_copy(identr[:], ident[:])
            ones16 = constp.tile([P, NK], F32, tag="ones16")
            nc.vector.memset(ones16[:], 1.0)

            state = {}

            def emit_load(g):
                b0 = g * GRP
                dg = dgp.tile([P, NK * N], BF16, tag="dg", bufs=8)
                rs = tinyp.tile([P, NK], F32, tag="rs", bufs=8)
                # diagonal per batch, partition-major -> w_pm[p, 4c+b]
                wpm = tinyp.tile([P, NK], F32, tag="wpm", bufs=10)
                for b in range(GRP):
                    diag_src = D_ap[b0 + b].rearrange("x y -> (x y)")[:: N + 1]
                    nc.sync.dma_start(
                        out=wpm[:][:, b : NK : GRP],
                        in_=diag_src.rearrange("(c p) -> p c", p=P),
                    )
                for b in range(GRP):
                    stg = stgp.tile([P, NCH * N], F32, tag="stg", bufs=6)
                    nc.sync.dma_start(
                        out=stg[:].rearrange("p (c j) -> p c j", c=NCH),
                        in_=D_ap[b0 + b].rearrange("(c p) j -> p c j", p=P),
                    )
                    # f32 -> bf16 convert; ACT is ~3x faster than GPSIMD,
                    # so split the four batches 3:1 between them.
                    dst = dg[:, b * NCH * N : (b + 1) * NCH * N]
                    if b == GRP - 1:
                        nc.gpsimd.tensor_copy(dst, stg[:])
                    else:
                        nc.scalar.copy(dst, stg[:])
                    # rowsum from the bf16 copy (16-bit input runs 2x faster
                    # on DVE; quantization error on a 512-sum is ~1e-4)
                    nc.vector.reduce_sum(
                        rs[:][:, b : NK : GRP],
                        dst.rearrange("p (c j) -> p c j", c=NCH),
                        axis=mybir.AxisListType.X,
                    )
                state[g] = {"dg": dg, "rs": rs, "wpm": wpm}

            def emit_prep(g):
                st = state[g]
                wpm = st["wpm"]
                # s = (1-w) / (rs - w + eps)
                num = tinyp.tile([P, NK], F32, tag="num", bufs=2)
                nc.vector.tensor_sub(num[:], ones16[:], wpm[:])
                den = tinyp.tile([P, NK], F32, tag="den", bufs=2)
                nc.vector.tensor_sub(den[:], st["rs"][:], wpm[:])
                nc.vector.tensor_scalar_add(den[:], den[:], EPS)
                rec = tinyp.tile([P, NK], F32, tag="rec", bufs=2)
                nc.vector.reciprocal(rec[:], den[:])
                spm = tinyp.tile([P, NK], F32, tag="spm", bufs=10)
                nc.vector.tensor_mul(spm[:], num[:], rec[:])
                # recC = 1/(1 + s*w) for the closing formula
                sw = tinyp.tile([P, NK], F32, tag="sw", bufs=2)
                nc.vector.tensor_mul(sw[:], spm[:], wpm[:])
                nc.vector.tensor_scalar_add(sw[:], sw[:], 1.0)
                recC = tinyp.tile([P, NK], F32, tag="recC", bufs=10)
                nc.vector.reciprocal(recC[:], sw[:])
                # Stationary ping-pong tiles [128, 48] bf16: cols 0..15 hold
                # d' for k=4c+b contiguously, cols 16..47 stay zero; the MM
                # stationary slab for k is ddp[:, k:k+32].
                ddA = cfmp.tile([P, NK + 32], BF16, tag="ddz", bufs=18)
                ddB = cfmp.tile([P, NK + 32], BF16, tag="ddz", bufs=18)
                nc.vector.memset(ddA[:], 0.0)
                nc.vector.memset(ddB[:], 0.0)
                # d'_0 = 1*s  (bf16)
                nc.vector.tensor_copy(ddA[:][:, 0:NK], spm[:])
                Sr = tinyp.tile([P, NK], F32, tag="Sr", bufs=10)
                st.update(spm=spm, recC=recC, dd=ddA, dd_nxt=ddB, Sr=Sr)

            def emit_iter(g, t):
                st = state[g]
                dg = st["dg"]
                dd = st["dd"]
                # m = dd*w_pm has no dependency on this iter's matmuls; emit
                # first so DVE computes it while the PE streams.
                if t < N_IT:
                    m = tinyp.tile([P, NK], F32, tag="m", bufs=6)
                    nc.vector.tensor_mul(m[:], dd[:][:, 0:NK], st["wpm"][:])
                Pt = psA.tile([P, N], F32, tag="P", bufs=5)
                for c in range(NCH):
                    for b in range(GRP):
                        k = NCH * c + b
                        nc.tensor.matmul(
                            Pt[32 * b : 32 * b + 32, :],
                            dd[:, k : k + 32],
                            dg[:, (b * NCH + c) * N : (b * NCH + c + 1) * N],
                            start=(c == 0),
                            stop=(c == NCH - 1),
                            tile_position=(0, 32 * b),
                            # the sim's group tracker drops partition bases
                            # and cannot model partition-disjoint groups in
                            # one bank; semantics are per-element has_written
                            skip_group_check=True,
                        )
                Ct = cfmp.tile([P, N], F32R, tag="C", bufs=6)
                nc.scalar.copy(Ct[:], Pt[:])
                pt = psT.tile([P, N], F32R, tag="pt", bufs=3)
                for jb in range(NCH):
                    nc.tensor.matmul(
                        pt[:, jb * P : (jb + 1) * P],
                        Ct[:, jb * P : (jb + 1) * P],
                        identr[:],
                        is_transpose=True,
                    )
                # compact the strided PSUM view once; everything after is
                # contiguous [128,16]
                Vc = tinyp.tile([P, NK], F32, tag="Vc", bufs=6)
                nc.vector.tensor_copy(Vc[:], pt[:][:, 0 : N : 32].bitcast(F32))
                if t == 1:
                    nc.vector.tensor_copy(st["Sr"][:], Vc[:])
                else:
                    nc.vector.tensor_add(st["Sr"][:], st["Sr"][:], Vc[:])
                if t < N_IT:
                    d = tinyp.tile([P, NK], F32, tag="d", bufs=6)
                    nc.vector.tensor_sub(d[:], Vc[:], m[:])
                    nxt = st["dd_nxt"]
                    nc.vector.tensor_mul(nxt[:][:, 0:NK], d[:], st["spm"][:])
                    st["dd_nxt"] = dd
                    st["dd"] = nxt

            def emit_close(g):
                st = state[g]
                b0 = g * GRP
                a = tinyp.tile([P, NK], F32, tag="m", bufs=6)
                nc.vector.tensor_scalar_add(a[:], st["Sr"][:], 1.0)
                b2 = tinyp.tile([P, NK], F32, tag="d", bufs=6)
                nc.vector.tensor_mul(b2[:], a[:], st["wpm"][:])
                o = tinyp.tile([P, NK], F32R, tag="o", bufs=2)
                nc.vector.tensor_mul(o[:], b2[:], st["recC"][:])
                po = psT.tile([P, N], F32R, tag="pt", bufs=3)
                nc.tensor.matmul(
                    po[0:NK, 0:P],
                    o[:],
                    identr[:],
                    is_transpose=True,
                )
                so = stoutp.tile([NK, P], F32, tag="so", bufs=3)
                nc.vector.tensor_copy(so[:], po[0:NK, 0:P].bitcast(F32))
                for c in range(NCH):
                    nc.sync.dma_start(
                        out=OUT_ap[b0 : b0 + GRP, c * P : (c + 1) * P],
                        in_=so[c * GRP : (c + 1) * GRP, :],
                    )

            # ---- staggered software pipeline over the 8 groups ----------
            first_slot = -LOAD_LEAD
            last_slot = (NGRP - 1) * STAG + N_IT
            for s in range(first_slot, last_slot + 1):
                for g in range(NGRP):
                    if s == g * STAG - LOAD_LEAD:
                        emit_load(g)
                for g in range(NGRP):
                    if s == g * STAG - 1:
                        emit_prep(g)
                for g in range(NGRP):
                    t = s - g * STAG + 1
                    if 1 <= t <= N_IT:
                        emit_iter(g, t)
                for g in range(NGRP):
                    if s == g * STAG + N_IT:
                        emit_close(g)
    return nc


_CACHED = None


def _build():
    global _CACHED
    if _CACHED is None:
        nc = bacc.Bacc(
            "TRN2", target_bir_lowering=False, debug=False, num_devices=1
        )
        _emit(nc)
        nc.compile()
        _CACHED = nc
    return _CACHED


def _run(D, **run_kwargs):
    nc = _build()
    D = np.ascontiguousarray(np.asarray(D, dtype=np.float32))
    assert D.shape == (B_TOTAL, N, N), D.shape
    in_maps = [
        {"D": D[i * B_CORE : (i + 1) * B_CORE]} for i in range(N_CORES)
    ]
    res = run_bass_kernel_spmd(nc, in_maps, core_ids=list(range(N_CORES)), **run_kwargs)
    out = np.concatenate([r["OUT"] for r in res.results], axis=0)
    return out, res


def kernel(D):
    out, _ = _run(D)
    return out


# revision 34
# speedup vs baseline: 2.3101x; 1.0982x over previous
"""Trainium2 Bass kernel for nn_Council_58050777972841.

Math per batch (n=512 citizens), D the raw delegation matrix:
    w  = diag(D);  rs = rowsum(D);  s = (1-w)/(rs-w+eps)
    iteration: d_{t+1} = d_t @ M,  M = diag(s)(D - diag(w))
    output   = d_N + w * sum_{t<N} d_t   (reference: N=100)

Identities used here:
  d_t @ M = (d_t*s) @ D - (d_t*s)*w        -> stream RAW D, no M precompute
  S := sum_{t=0..N-1} d_t  satisfies  S*(1+s*w) = S_r + 1 - d_N
       where S_r = sum_{t=1..N} r_t and r_t = (d_{t-1}*s) @ D
  out = w*S + d_N  ~=  w*(S_r+1)/(1+s*w)   (d_N dropped; |d_N|~0.5^N)

N_IT=10 gives ~7e-4 max rel error vs the 100-iter reference (incl. bf16
quantization of the streamed D), far inside the 2e-2 gate.

Layout per core (32 batches, groups of 4):
  D group tile: bf16 [128, 4b*4c*512], partition p = i%128, chunk c = i//128.
  matvec: 4 batches concurrently via PE column tiles tile_position=(0,32b).
     Stationary for (b,c) is ddp[:, k:k+32] (k=4c+b) of the zero-padded
     [128,48] tile whose cols 0..15 hold d' contiguously — so every MM
     writes a full 32-row PSUM slab (rows beyond 32b are never read).
  r_t: PSUM bank [128,512] -> ACT copy (f32r) -> 4 PE transposes -> compact
     V [128,16]; DVE chain: S_r+=V; dd' = (V - dd*w_pm)*s_pm  (bf16).
  closing: out_pm = w*(S_r+1)/(1+s*w) -> 1 transpose -> 4 small DMAs to OUT.
Groups are software-pipelined on a 2-iteration stagger so each group's
per-iteration dependency gap is covered by other groups' matmuls; loads
(f32 staging DMA) + bf16 converts (ACT 3:1 GPSIMD) run several slots ahead.
"""

import sys

if "/opt/trn_rl_repo" not in sys.path:
    sys.path.insert(0, "/opt/trn_rl_repo")

import os
import numpy as np

import concourse.bacc as bacc
import concourse.mybir as mybir
from concourse import masks
from concourse.tile import TileContext
from concourse.bass_utils import run_bass_kernel_spmd

P = 128
N = 512
NCH = 4          # i-chunks of 128
GRP = 4          # batches per group (= PE column tiles)
NK = GRP * NCH   # 16 (b,c) pairs per group
N_CORES = 8
B_TOTAL = 256
B_CORE = B_TOTAL // N_CORES
NGRP = B_CORE // GRP
STAG = 2         # iteration-slot stagger between consecutive groups
LOAD_LEAD = 4    # slots between a group's load and its first iteration
N_IT = int(os.environ.get("COUNCIL_N_IT", "10"))
EPS = 1e-6

F32 = mybir.dt.float32
F32R = mybir.dt.float32r
BF16 = mybir.dt.bfloat16
ALU = mybir.AluOpType


def _emit(nc):
    D_dram = nc.dram_tensor("D", [B_CORE, N, N], F32, kind="ExternalInput")
    OUT_dram = nc.dram_tensor("OUT", [B_CORE, N], F32, kind="ExternalOutput")
    D_ap = D_dram.ap()
    OUT_ap = OUT_dram.ap()

    with TileContext(nc) as tc:
        with (
            tc.tile_pool(name="const", bufs=1) as constp,
            tc.tile_pool(name="stg", bufs=1) as stgp,
            tc.tile_pool(name="dgp", bufs=1) as dgp,
            tc.tile_pool(name="cfm", bufs=1) as cfmp,
            tc.tile_pool(name="tiny", bufs=1) as tinyp,
            tc.tile_pool(name="stout", bufs=1) as stoutp,
            tc.tile_pool(name="psA", bufs=1, space="PSUM") as psA,
            tc.tile_pool(name="psT", bufs=1, space="PSUM") as psT,
        ):
            ident = constp.tile([P, P], F32, tag="ident")
            masks.make_identity(nc, ident[:])
            identr = constp.tile([P, P], F32R, tag="identr")
            nc.vector.tensor_copy(identr[:], ident[:])
            identb = constp.tile([P, P], BF16, tag="identb")
            nc.vector.tensor_copy(identb[:], ident[:])
            ones16 = constp.tile([P, NK], F32, tag="ones16")
            nc.vector.memset(ones16[:], 1.0)

            state = {}

            def emit_load(g):
                b0 = g * GRP
                dg = dgp.tile([P, NK * N], BF16, tag="dg", bufs=8)
                rs = tinyp.tile([P, NK], F32, tag="rs", bufs=8)
                # diagonal per batch, partition-major -> w_pm[p, 4c+b]
                wpm = tinyp.tile([P, NK], F32, tag="wpm", bufs=10)
                for b in range(GRP):
                    stg = stgp.tile([P, NCH * N], F32, tag="stg", bufs=6)
                    nc.sync.dma_start(
                        out=stg[:].rearrange("p (c j) -> p c j", c=NCH),
                        in_=D_ap[b0 + b].rearrange("(c p) j -> p c j", p=P),
                    )
                    # f32 -> bf16 convert, spread over engines: first two
                    # groups alternate ACT/DVE (DVE is idle at startup and
                    # fastest); steady state runs 3:1 ACT:GPSIMD.
                    dst = dg[:, b * NCH * N : (b + 1) * NCH * N]
                    if g < 2:
                        if b % 2 == 0:
                            nc.scalar.copy(dst, stg[:])
                        else:
                            nc.vector.tensor_copy(dst, stg[:])
                    elif b == GRP - 1:
                        nc.gpsimd.tensor_copy(dst, stg[:])
                    else:
                        nc.scalar.copy(dst, stg[:])
                    # rowsum from the bf16 copy (16-bit input runs 2x faster
                    # on DVE; quantization error on a 512-sum is ~1e-4)
                    nc.vector.reduce_sum(
                        rs[:][:, b : NK : GRP],
                        dst.rearrange("p (c j) -> p c j", c=NCH),
                        axis=mybir.AxisListType.X,
                    )
                # diag DMAs are descriptor-heavy (512 x 4B); emit them after
                # the bulk loads so they don't head-of-line-block the queues
                for b in range(GRP):
                    diag_src = D_ap[b0 + b].rearrange("x y -> (x y)")[:: N + 1]
                    nc.sync.dma_start(
                        out=wpm[:][:, b : NK : GRP],
                        in_=diag_src.rearrange("(c p) -> p c", p=P),
                    )
                state[g] = {"dg": dg, "rs": rs, "wpm": wpm}

            def emit_prep(g):
                st = state[g]
                wpm = st["wpm"]
                # s = (1-w) / (rs - w + eps)
                num = tinyp.tile([P, NK], F32, tag="num", bufs=2)
                nc.vector.tensor_sub(num[:], ones16[:], wpm[:])
                den = tinyp.tile([P, NK], F32, tag="den", bufs=2)
                nc.vector.tensor_sub(den[:], st["rs"][:], wpm[:])
                nc.vector.tensor_scalar_add(den[:], den[:], EPS)
                rec = tinyp.tile([P, NK], F32, tag="rec", bufs=2)
                nc.vector.reciprocal(rec[:], den[:])
                spm = tinyp.tile([P, NK], F32, tag="spm", bufs=10)
                nc.vector.tensor_mul(spm[:], num[:], rec[:])
                # recC = 1/(1 + s*w) for the closing formula
                sw = tinyp.tile([P, NK], F32, tag="sw", bufs=2)
                nc.vector.tensor_mul(sw[:], spm[:], wpm[:])
                nc.vector.tensor_scalar_add(sw[:], sw[:], 1.0)
                recC = tinyp.tile([P, NK], F32, tag="recC", bufs=10)
                nc.vector.reciprocal(recC[:], sw[:])
                # Stationary ping-pong tiles [128, 48] bf16: cols 0..15 hold
                # d' for k=4c+b contiguously, cols 16..47 stay zero; the MM
                # stationary slab for k is ddp[:, k:k+32].
                ddA = cfmp.tile([P, NK + 32], BF16, tag="ddz", bufs=18)
                ddB = cfmp.tile([P, NK + 32], BF16, tag="ddz", bufs=18)
                nc.vector.memset(ddA[:], 0.0)
                nc.vector.memset(ddB[:], 0.0)
                # d'_0 = 1*s  (bf16)
                nc.vector.tensor_copy(ddA[:][:, 0:NK], spm[:])
                Sr = tinyp.tile([P, NK], F32, tag="Sr", bufs=10)
                st.update(spm=spm, recC=recC, dd=ddA, dd_nxt=ddB, Sr=Sr)

            def emit_iter(g, t):
                st = state[g]
                dg = st["dg"]
                dd = st["dd"]
                # m = dd*w_pm has no dependency on this iter's matmuls; emit
                # first so DVE computes it while the PE streams.
                if t < N_IT:
                    m = tinyp.tile([P, NK], F32, tag="m", bufs=6)
                    nc.vector.tensor_mul(m[:], dd[:][:, 0:NK], st["wpm"][:])
                Pt = psA.tile([P, N], F32, tag="P", bufs=5)
                for c in range(NCH):
                    for b in range(GRP):
                        k = NCH * c + b
                        nc.tensor.matmul(
                            Pt[32 * b : 32 * b + 32, :],
                            dd[:, k : k + 32],
                            dg[:, (b * NCH + c) * N : (b * NCH + c + 1) * N],
                            start=(c == 0),
                            stop=(c == NCH - 1),
                            tile_position=(0, 32 * b),
                            # the sim's group tracker drops partition bases
                            # and cannot model partition-disjoint groups in
                            # one bank; semantics are per-element has_written
                            skip_group_check=True,
                        )
                Ct = cfmp.tile([P, N], BF16, tag="C", bufs=6)
                nc.scalar.copy(Ct[:], Pt[:])
                pt = psT.tile([P, N], BF16, tag="pt", bufs=2)
                for jb in range(NCH):
                    nc.tensor.matmul(
                        pt[:, jb * P : (jb + 1) * P],
                        Ct[:, jb * P : (jb + 1) * P],
                        identb[:],
                        is_transpose=True,
                    )
                # compact the strided PSUM view once; everything after is
                # contiguous [128,16]
                Vc = tinyp.tile([P, NK], F32, tag="Vc", bufs=6)
                nc.vector.tensor_copy(Vc[:], pt[:][:, 0 : N : 32])
                if t == 1:
                    nc.vector.tensor_copy(st["Sr"][:], Vc[:])
                else:
                    nc.vector.tensor_add(st["Sr"][:], st["Sr"][:], Vc[:])
                if t < N_IT:
                    d = tinyp.tile([P, NK], F32, tag="d", bufs=6)
                    nc.vector.tensor_sub(d[:], Vc[:], m[:])
                    nxt = st["dd_nxt"]
                    nc.vector.tensor_mul(nxt[:][:, 0:NK], d[:], st["spm"][:])
                    st["dd_nxt"] = dd
                    st["dd"] = nxt

            def emit_close(g):
                st = state[g]
                b0 = g * GRP
                a = tinyp.tile([P, NK], F32, tag="m", bufs=6)
                nc.vector.tensor_scalar_add(a[:], st["Sr"][:], 1.0)
                b2 = tinyp.tile([P, NK], F32, tag="d", bufs=6)
                nc.vector.tensor_mul(b2[:], a[:], st["wpm"][:])
                o = tinyp.tile([P, NK], F32R, tag="o", bufs=2)
                nc.vector.tensor_mul(o[:], b2[:], st["recC"][:])
                po = psT.tile([P, N], F32R, tag="po", bufs=1)
                nc.tensor.matmul(
                    po[0:NK, 0:P],
                    o[:],
                    identr[:],
                    is_transpose=True,
                )
                so = stoutp.tile([NK, P], F32, tag="so", bufs=3)
                nc.vector.tensor_copy(so[:], po[0:NK, 0:P].bitcast(F32))
                for c in range(NCH):
                    nc.sync.dma_start(
                        out=OUT_ap[b0 : b0 + GRP, c * P : (c + 1) * P],
                        in_=so[c * GRP : (c + 1) * GRP, :],
                    )

            # ---- staggered software pipeline over the 8 groups ----------
            first_slot = -LOAD_LEAD
            last_slot = (NGRP - 1) * STAG + N_IT
            for s in range(first_slot, last_slot + 1):
                for g in range(NGRP):
                    if s == g * STAG - LOAD_LEAD:
                        emit_load(g)
                for g in range(NGRP):
                    if s == g * STAG - 1:
                        emit_prep(g)
                for g in range(NGRP):
                    t = s - g * STAG + 1
                    if 1 <= t <= N_IT:
                        emit_iter(g, t)
                for g in range(NGRP):
                    if s == g * STAG + N_IT:
                        emit_close(g)
    return nc


_CACHED = None


def _build():
    global _CACHED
    if _CACHED is None:
        nc = bacc.Bacc(
            "TRN2", target_bir_lowering=False, debug=False, num_devices=1
        )
        _emit(nc)
        nc.compile()
        _CACHED = nc
    return _CACHED


def _run(D, **run_kwargs):
    nc = _build()
    D = np.ascontiguousarray(np.asarray(D, dtype=np.float32))
    assert D.shape == (B_TOTAL, N, N), D.shape
    in_maps = [
        {"D": D[i * B_CORE : (i + 1) * B_CORE]} for i in range(N_CORES)
    ]
    res = run_bass_kernel_spmd(nc, in_maps, core_ids=list(range(N_CORES)), **run_kwargs)
    out = np.concatenate([r["OUT"] for r in res.results], axis=0)
    return out, res


def kernel(D):
    out, _ = _run(D)
    return out


# revision 35
# speedup vs baseline: 2.5678x; 1.1115x over previous
"""Trainium2 Bass kernel for nn_Council_58050777972841.

Math per batch (n=512 citizens), D the raw delegation matrix:
    w  = diag(D);  rs = rowsum(D);  s = (1-w)/(rs-w+eps)
    iteration: d_{t+1} = d_t @ M,  M = diag(s)(D - diag(w))
    output   = d_N + w * sum_{t<N} d_t   (reference: N=100)

Identities used here:
  d_t @ M = (d_t*s) @ D - (d_t*s)*w        -> stream RAW D, no M precompute
  S := sum_{t=0..N-1} d_t  satisfies  S*(1+s*w) = S_r + 1 - d_N
       where S_r = sum_{t=1..N} r_t and r_t = (d_{t-1}*s) @ D
  out = w*S + d_N  ~=  w*(S_r+1)/(1+s*w)   (d_N dropped; |d_N|~0.5^N)

N_IT=10 gives ~7e-4 max rel error vs the 100-iter reference (incl. bf16
quantization of the streamed D), far inside the 2e-2 gate.

Layout per core (32 batches, groups of 4):
  D group tile: bf16 [128, 4b*4c*512], partition p = i%128, chunk c = i//128.
  matvec: 4 batches concurrently via PE column tiles tile_position=(0,32b).
     Stationary for (b,c) is ddp[:, k:k+32] (k=4c+b) of the zero-padded
     [128,48] tile whose cols 0..15 hold d' contiguously — so every MM
     writes a full 32-row PSUM slab (rows beyond 32b are never read).
  r_t: PSUM bank [128,512] -> ACT copy (f32r) -> 4 PE transposes -> compact
     V [128,16]; DVE chain: S_r+=V; dd' = (V - dd*w_pm)*s_pm  (bf16).
  closing: out_pm = w*(S_r+1)/(1+s*w) -> 1 transpose -> 4 small DMAs to OUT.
Groups are software-pipelined on a 2-iteration stagger so each group's
per-iteration dependency gap is covered by other groups' matmuls; loads
(f32 staging DMA) + bf16 converts (ACT 3:1 GPSIMD) run several slots ahead.
"""

import sys

if "/opt/trn_rl_repo" not in sys.path:
    sys.path.insert(0, "/opt/trn_rl_repo")

import os
import numpy as np

import concourse.bacc as bacc
import concourse.mybir as mybir
from concourse import masks
from concourse.tile import TileContext
from concourse.bass_utils import run_bass_kernel_spmd

P = 128
N = 512
NCH = 4          # i-chunks of 128
GRP = 4          # batches per group (= PE column tiles)
NK = GRP * NCH   # 16 (b,c) pairs per group
N_CORES = 8
B_TOTAL = 256
B_CORE = B_TOTAL // N_CORES
NGRP = B_CORE // GRP
STAG = 2         # iteration-slot stagger between consecutive groups
LOAD_LEAD = 4    # slots between a group's load and its first iteration
N_IT = int(os.environ.get("COUNCIL_N_IT", "10"))
EPS = 1e-6

F32 = mybir.dt.float32
F32R = mybir.dt.float32r
BF16 = mybir.dt.bfloat16
ALU = mybir.AluOpType


def _emit(nc):
    D_dram = nc.dram_tensor("D", [B_CORE, N, N], F32, kind="ExternalInput")
    OUT_dram = nc.dram_tensor("OUT", [B_CORE, N], F32, kind="ExternalOutput")
    D_ap = D_dram.ap()
    OUT_ap = OUT_dram.ap()

    with TileContext(nc) as tc:
        with (
            tc.tile_pool(name="const", bufs=1) as constp,
            tc.tile_pool(name="stg", bufs=1) as stgp,
            tc.tile_pool(name="dgp", bufs=1) as dgp,
            tc.tile_pool(name="cfm", bufs=1) as cfmp,
            tc.tile_pool(name="tiny", bufs=1) as tinyp,
            tc.tile_pool(name="stout", bufs=1) as stoutp,
            tc.tile_pool(name="psA", bufs=1, space="PSUM") as psA,
            tc.tile_pool(name="psT", bufs=1, space="PSUM") as psT,
        ):
            ident = constp.tile([P, P], F32, tag="ident")
            masks.make_identity(nc, ident[:])
            identr = constp.tile([P, P], F32R, tag="identr")
            nc.vector.tensor_copy(identr[:], ident[:])
            identb = constp.tile([P, P], BF16, tag="identb")
            nc.vector.tensor_copy(identb[:], ident[:])
            ones16 = constp.tile([P, NK], F32, tag="ones16")
            nc.vector.memset(ones16[:], 1.0)

            state = {}

            def emit_load(g):
                b0 = g * GRP
                dg = dgp.tile([P, NK * N], BF16, tag="dg", bufs=8)
                rs = tinyp.tile([P, NK], F32, tag="rs", bufs=8)
                # diagonal per batch, partition-major -> w_pm[p, 4c+b]
                wpm = tinyp.tile([P, NK], F32, tag="wpm", bufs=10)
                for b in range(GRP):
                    stg = stgp.tile([P, NCH * N], F32, tag="stg", bufs=6)
                    nc.sync.dma_start(
                        out=stg[:].rearrange("p (c j) -> p c j", c=NCH),
                        in_=D_ap[b0 + b].rearrange("(c p) j -> p c j", p=P),
                    )
                    # f32 -> bf16 convert, spread over engines: first two
                    # groups alternate ACT/DVE (DVE is idle at startup and
                    # fastest); steady state runs 3:1 ACT:GPSIMD.
                    dst = dg[:, b * NCH * N : (b + 1) * NCH * N]
                    if g < 2:
                        if b % 2 == 0:
                            nc.scalar.copy(dst, stg[:])
                        else:
                            nc.vector.tensor_copy(dst, stg[:])
                    elif b == GRP - 1:
                        nc.gpsimd.tensor_copy(dst, stg[:])
                    else:
                        nc.scalar.copy(dst, stg[:])
                    # rowsum from the bf16 copy (16-bit input runs 2x faster
                    # on DVE; quantization error on a 512-sum is ~1e-4)
                    nc.vector.reduce_sum(
                        rs[:][:, b : NK : GRP],
                        dst.rearrange("p (c j) -> p c j", c=NCH),
                        axis=mybir.AxisListType.X,
                    )
                # diag DMAs are descriptor-heavy (512 x 4B); emit them after
                # the bulk loads so they don't head-of-line-block the queues
                for b in range(GRP):
                    diag_src = D_ap[b0 + b].rearrange("x y -> (x y)")[:: N + 1]
                    dsrc = diag_src.rearrange("(c p) -> p c", p=P)
                    nc.sync.dma_start(
                        out=wpm[0:64, b : NK : GRP], in_=dsrc[0:64]
                    )
                    nc.sync.dma_start(
                        out=wpm[64:P, b : NK : GRP], in_=dsrc[64:P]
                    )
                state[g] = {"dg": dg, "rs": rs, "wpm": wpm}

            def emit_prep(g):
                st = state[g]
                wpm = st["wpm"]
                # s = (1-w) / (rs - w + eps)
                num = tinyp.tile([P, NK], F32, tag="num", bufs=2)
                nc.vector.tensor_sub(num[:], ones16[:], wpm[:])
                den = tinyp.tile([P, NK], F32, tag="den", bufs=2)
                nc.vector.tensor_sub(den[:], st["rs"][:], wpm[:])
                nc.vector.tensor_scalar_add(den[:], den[:], EPS)
                rec = tinyp.tile([P, NK], F32, tag="rec", bufs=2)
                nc.vector.reciprocal(rec[:], den[:])
                spm = tinyp.tile([P, NK], F32, tag="spm", bufs=10)
                nc.vector.tensor_mul(spm[:], num[:], rec[:])
                # recC = 1/(1 + s*w) for the closing formula
                sw = tinyp.tile([P, NK], F32, tag="sw", bufs=2)
                nc.vector.tensor_mul(sw[:], spm[:], wpm[:])
                nc.vector.tensor_scalar_add(sw[:], sw[:], 1.0)
                recC = tinyp.tile([P, NK], F32, tag="recC", bufs=10)
                nc.vector.reciprocal(recC[:], sw[:])
                # Stationary ping-pong tiles [128, 48] bf16: cols 0..15 hold
                # d' for k=4c+b contiguously, cols 16..47 stay zero; the MM
                # stationary slab for k is ddp[:, k:k+32].
                ddA = cfmp.tile([P, NK + 32], BF16, tag="ddz", bufs=18)
                ddB = cfmp.tile([P, NK + 32], BF16, tag="ddz", bufs=18)
                nc.vector.memset(ddA[:], 0.0)
                nc.vector.memset(ddB[:], 0.0)
                # d'_0 = 1*s  (bf16)
                nc.vector.tensor_copy(ddA[:][:, 0:NK], spm[:])
                Sr = tinyp.tile([P, NK], F32, tag="Sr", bufs=10)
                st.update(spm=spm, recC=recC, dd=ddA, dd_nxt=ddB, Sr=Sr)

            def emit_iter(g, t):
                st = state[g]
                dg = st["dg"]
                dd = st["dd"]
                # m = dd*w_pm has no dependency on this iter's matmuls; emit
                # first so DVE computes it while the PE streams.
                if t < N_IT:
                    m = tinyp.tile([P, NK], F32, tag="m", bufs=6)
                    nc.gpsimd.tensor_mul(m[:], dd[:][:, 0:NK], st["wpm"][:])
                Pt = psA.tile([P, N], F32, tag="P", bufs=5)
                for c in range(NCH):
                    for b in range(GRP):
                        k = NCH * c + b
                        nc.tensor.matmul(
                            Pt[32 * b : 32 * b + 32, :],
                            dd[:, k : k + 32],
                            dg[:, (b * NCH + c) * N : (b * NCH + c + 1) * N],
                            start=(c == 0),
                            stop=(c == NCH - 1),
                            tile_position=(0, 32 * b),
                            # the sim's group tracker drops partition bases
                            # and cannot model partition-disjoint groups in
                            # one bank; semantics are per-element has_written
                            skip_group_check=True,
                        )
                Ct = cfmp.tile([P, N], BF16, tag="C", bufs=6)
                nc.scalar.copy(Ct[:], Pt[:])
                pt = psT.tile([P, N], BF16, tag="pt", bufs=2)
                for jb in range(NCH):
                    nc.tensor.matmul(
                        pt[:, jb * P : (jb + 1) * P],
                        Ct[:, jb * P : (jb + 1) * P],
                        identb[:],
                        is_transpose=True,
                    )
                # compact the strided PSUM view once; everything after is
                # contiguous [128,16]
                Vc = tinyp.tile([P, NK], F32, tag="Vc", bufs=6)
                nc.vector.tensor_copy(Vc[:], pt[:][:, 0 : N : 32])
                if t == 1:
                    nc.gpsimd.tensor_copy(st["Sr"][:], Vc[:])
                else:
                    nc.gpsimd.tensor_add(st["Sr"][:], st["Sr"][:], Vc[:])
                if t < N_IT:
                    d = tinyp.tile([P, NK], F32, tag="d", bufs=6)
                    nc.vector.tensor_sub(d[:], Vc[:], m[:])
                    nxt = st["dd_nxt"]
                    nc.vector.tensor_mul(nxt[:][:, 0:NK], d[:], st["spm"][:])
                    st["dd_nxt"] = dd
                    st["dd"] = nxt

            def emit_close(g):
                st = state[g]
                b0 = g * GRP
                a = tinyp.tile([P, NK], F32, tag="m", bufs=6)
                nc.gpsimd.tensor_scalar_add(a[:], st["Sr"][:], 1.0)
                b2 = tinyp.tile([P, NK], F32, tag="d", bufs=6)
                nc.gpsimd.tensor_mul(b2[:], a[:], st["wpm"][:])
                o = tinyp.tile([P, NK], F32R, tag="o", bufs=2)
                nc.vector.tensor_mul(o[:], b2[:], st["recC"][:])
                po = psT.tile([P, N], F32R, tag="po", bufs=1)
                nc.tensor.matmul(
                    po[0:NK, 0:P],
                    o[:],
                    identr[:],
                    is_transpose=True,
                )
                so = stoutp.tile([NK, P], F32, tag="so", bufs=3)
                nc.vector.tensor_copy(so[:], po[0:NK, 0:P].bitcast(F32))
                for c in range(NCH):
                    nc.sync.dma_start(
                        out=OUT_ap[b0 : b0 + GRP, c * P : (c + 1) * P],
                        in_=so[c * GRP : (c + 1) * GRP, :],
                    )

            # ---- staggered software pipeline over the 8 groups ----------
            first_slot = -LOAD_LEAD
            last_slot = (NGRP - 1) * STAG + N_IT
            for s in range(first_slot, last_slot + 1):
                for g in range(NGRP):
                    if s == g * STAG - LOAD_LEAD:
                        emit_load(g)
                for g in range(NGRP):
                    if s == g * STAG - 1:
                        emit_prep(g)
                for g in range(NGRP):
                    t = s - g * STAG + 1
                    if 1 <= t <= N_IT:
                        emit_iter(g, t)
                for g in range(NGRP):
                    if s == g * STAG + N_IT:
                        emit_close(g)
    return nc


_CACHED = None


def _build():
    global _CACHED
    if _CACHED is None:
        nc = bacc.Bacc(
            "TRN2", target_bir_lowering=False, debug=False, num_devices=1
        )
        _emit(nc)
        nc.compile()
        _CACHED = nc
    return _CACHED


def _run(D, **run_kwargs):
    nc = _build()
    D = np.ascontiguousarray(np.asarray(D, dtype=np.float32))
    assert D.shape == (B_TOTAL, N, N), D.shape
    in_maps = [
        {"D": D[i * B_CORE : (i + 1) * B_CORE]} for i in range(N_CORES)
    ]
    res = run_bass_kernel_spmd(nc, in_maps, core_ids=list(range(N_CORES)), **run_kwargs)
    out = np.concatenate([r["OUT"] for r in res.results], axis=0)
    return out, res


def kernel(D):
    out, _ = _run(D)
    return out
